# revision 42
# baseline (speedup 1.0000x reference)
"""Trainium2 Bass kernel for nn_AggregatedBilinear.

Computation (per batch row b):
    x1 = ELU(input1 @ W1.T)                    # [128]
    x2 = ELU(input2 @ W2.T)                    # [128]
    y[g,o] = sum_ij x1[g,i] Wb[g,o,i,j] x2[g,j]   (g<32, o,i,j<4)
    out = ELU(y) @ Wout.T                      # [512]

Strategy: data-parallel over 8 NeuronCores (8192 batch rows each). On-chip
layout is feature-major (features on SBUF partitions, batch on the free dim),
so the host pre-transposes each input shard (fp32 DMA-transpose does not
exist on trn2) and transposes the result back. All DMA'd tensors are bf16
(halves HBM traffic; the correctness budget is rel RMS < 2e-2).

The per-group bilinear uses a rank-8 CP decomposition (exact for these
4x4x4 tensors):
    Wb[g,o,i,j] = sum_r C[g,o,r] U[g,i,r] V[g,j,r]
    tt[(g,r)]   = (U.T a1)[(g,r)] * (V.T a2)[(g,r)]    # PE + DVE multiply
    y           = Wc.T @ tt                             # PE, 2 chunks of 128

ELU is computed in the shifted form a' = ELU(x)+1 = relu(x) + min(exp(x),1)
(min(exp(x),1) == exp(min(x,0)) including exp overflow to +inf, so the
unclamped ACT Exp is safe).  The +1 shifts are folded into constants:
ttf = ru - su with su = U.T 1, tt = (rv - sv)*ttf, and the fc_out bias
bout = -Wout @ 1.  Engine assignment per batch tile keeps every engine
under the ~76us DMA floor: ACT does the Exps + ru staging, GpSimd (no
PSUM port, so SBUF-only ops) does the min's, DVE does the 2-stream
merges, and the PSUM->SBUF output converts alternate ACT/DVE.
"""

import numpy as np

B = 65536
IN1 = IN2 = 512
OUT = 512
CARD = 32
WIDTH = 4
INTERNAL = CARD * WIDTH  # 128
N_CORES = 8
B_CORE = B // N_CORES  # 8192
NT = 512  # batch columns per tile (one PSUM bank)
N_TILES = B_CORE // NT  # 16

_CACHE = {}


def _ensure_path():
    import sys

    try:
        import concourse  # noqa: F401
    except ImportError:
        for p in ("/opt/trn_rl_repo", "/root/.axon_site/_ro/trn_rl_repo"):
            if p not in sys.path:
                sys.path.insert(0, p)


def _split_excess_waits(nc, max_waits=1):
    """walrus CoreV3 codegen rejects instructions with more than a couple of
    semaphore waits; split excess waits onto preceding NoOps."""
    from concourse import mybir

    n_new = 0
    for f in nc.m.functions:
        for bb in f.blocks:
            insts = list(bb.instructions)
            out = []
            changed = False
            for inst in insts:
                si = inst.sync_info
                if si is not None and si.on_wait and len(si.on_wait) > max_waits:
                    waits = list(si.on_wait)
                    excess, keep = waits[:-max_waits], waits[-max_waits:]
                    for i in range(0, len(excess), max_waits):
                        nop = mybir.InstNoOp(
                            name=f"waitsplit-{n_new}",
                            engine=inst.engine,
                            ins=[],
                            outs=[],
                            sync_info=mybir.SyncInfo(
                                on_wait=excess[i : i + max_waits], on_update=[]
                            ),
                        )
                        n_new += 1
                        out.append(nop)
                    inst.sync_info = mybir.SyncInfo(
                        on_wait=keep, on_update=list(si.on_update or [])
                    )
                    changed = True
                out.append(inst)
            if changed:
                bb.instructions[:] = out
    return n_new


DEFAULT_CFG = dict(
    in_db=2, out_db=2, bufs_in=2, bufs_pf=1, bufs_pr=2, bufs_py=2,
    bufs_t=4, bufs_ye=3, bufs_out=2,
    io_dtype="float16", cp_dtype="float16", out_dtype="float16",
)


def _build_program(reps=1, loop_reps=None, cfg=None):
    """loop_reps: wrap the whole batch sweep in an on-device For_i that runs
    it loop_reps times (same I/O; used only for timing measurements)."""
    cfg = dict(DEFAULT_CFG, **(cfg or {}))
    import concourse.bass as bass
    import concourse.tile as tile
    from concourse import mybir

    f32 = mybir.dt.float32
    # idt: dtype of the DMA'd inputs + fc1/fc2/fc_out matmul operands.
    # mdt: dtype of the CP-stage matmul operands (tt and eu/ev/wc).
    # odt: dtype of the DMA'd output.
    idt = getattr(mybir.dt, cfg["io_dtype"])
    mdt = getattr(mybir.dt, cfg["cp_dtype"])
    odt = getattr(mybir.dt, cfg["out_dtype"])
    Exp = mybir.ActivationFunctionType.Exp
    Alu = mybir.AluOpType

    nc = bass.Bass()
    x1t = nc.declare_dram_parameter("x1t", [IN1, B_CORE], idt, isOutput=False)
    x2t = nc.declare_dram_parameter("x2t", [IN2, B_CORE], idt, isOutput=False)
    w1t = nc.declare_dram_parameter("w1t", [4, 128, 128], idt, isOutput=False)
    w2t = nc.declare_dram_parameter("w2t", [4, 128, 128], idt, isOutput=False)
    woutt = nc.declare_dram_parameter("woutt", [4, 128, 128], idt, isOutput=False)
    eu = nc.declare_dram_parameter("eu", [2, 128, 128], mdt, isOutput=False)
    ev = nc.declare_dram_parameter("ev", [2, 128, 128], mdt, isOutput=False)
    wc = nc.declare_dram_parameter("wc", [2, 128, 128], mdt, isOutput=False)
    suc = nc.declare_dram_parameter("suc", [128, 2], f32, isOutput=False)
    svc = nc.declare_dram_parameter("svc", [128, 2], f32, isOutput=False)
    bout = nc.declare_dram_parameter("bout", [128, 4], f32, isOutput=False)
    outt = nc.declare_dram_parameter("outt", [OUT, B_CORE], odt, isOutput=True)

    with tile.TileContext(nc) as tc:
        with (
            tc.tile_pool(name="consts", bufs=1) as consts,
            tc.tile_pool(name="inp", bufs=cfg["bufs_in"]) as pool_in,
            tc.tile_pool(name="inp0", bufs=1) as pool_in0,
            tc.tile_pool(name="tmp", bufs=2) as pool_tmp,
            tc.tile_pool(name="act", bufs=2) as pool_a,
            tc.tile_pool(name="tmul", bufs=cfg["bufs_t"]) as pool_t,
            tc.tile_pool(name="yep", bufs=cfg["bufs_ye"]) as pool_ye,
            tc.tile_pool(name="outs", bufs=cfg["bufs_out"]) as pool_out,
            tc.tile_pool(name="pfc", bufs=cfg["bufs_pf"], space="PSUM") as pool_pf,
            tc.tile_pool(name="prep", bufs=cfg["bufs_pr"], space="PSUM") as pool_pr,
            tc.tile_pool(name="py", bufs=cfg["bufs_py"], space="PSUM") as pool_py,
        ):
            # --- first input group DMAs issue before the (many) const DMAs
            # so the x transfers start streaming immediately.  The first
            # sub's two tiles go first as small separate transfers so fc(0)
            # can start after ~1/G of the group transfer time. ---
            G = cfg["in_db"]
            x1v0 = x1t.rearrange("(c k) b -> k c b", k=128)
            x2v0 = x2t.rearrange("(c k) b -> k c b", k=128)
            x1sb0 = pool_in0.tile([128, 4, G * NT], idt, tag="x10")
            x2sb0 = pool_in0.tile([128, 4, G * NT], idt, tag="x20")
            for sub in range(G):
                sl = bass.ds(sub * NT, NT)
                nc.sync.dma_start(out=x1sb0[:, :, sl], in_=x1v0[:, :, sl])
                nc.sync.dma_start(out=x2sb0[:, :, sl], in_=x2v0[:, :, sl])

            # --- constants into SBUF ---
            def load_const3(h, dt, nchunk=4):
                t = consts.tile([128, nchunk, 128], dt, tag=h.name)
                nc.sync.dma_start(out=t, in_=h.rearrange("c k m -> k c m"))
                return t

            w1sb = load_const3(w1t, idt)
            w2sb = load_const3(w2t, idt)
            woutsb = load_const3(woutt, idt)
            eusb = load_const3(eu, mdt, 2)
            evsb = load_const3(ev, mdt, 2)
            wcsb = load_const3(wc, mdt, 2)
            susb = consts.tile([128, 2], f32, tag="suc")
            nc.sync.dma_start(out=susb, in_=suc[:, :])
            svsb = consts.tile([128, 2], f32, tag="svc")
            nc.sync.dma_start(out=svsb, in_=svc[:, :])
            boutsb = consts.tile([128, 4], f32, tag="bout")
            nc.sync.dma_start(out=boutsb, in_=bout[:, :])

            x1v = x1t.rearrange("(c k) b -> k c b", k=128)
            x2v = x2t.rearrange("(c k) b -> k c b", k=128)
            outv = outt.rearrange("(c k) b -> k c b", k=128)

            import contextlib

            loop_cm = (
                tc.For_i(0, loop_reps, 1)
                if loop_reps is not None
                else contextlib.nullcontext()
            )
            with loop_cm:
                _batch_sweep(
                    nc, tc, reps, f32, (idt, mdt, odt), Exp, Alu,
                    x1v, x2v, outv,
                    w1sb, w2sb, woutsb, eusb, evsb, wcsb,
                    susb, svsb, boutsb,
                    pool_in, pool_tmp, pool_a, pool_t, pool_ye, pool_out,
                    pool_pf, pool_pr, pool_py, cfg,
                    (x1sb0, x2sb0),
                )

    _split_excess_waits(nc)
    return nc


def _batch_sweep(
    nc, tc, reps, f32, dts, Exp, Alu,
    x1v, x2v, outv,
    w1sb, w2sb, woutsb, eusb, evsb, wcsb,
    susb, svsb, boutsb,
    pool_in, pool_tmp, pool_a, pool_t, pool_ye, pool_out,
    pool_pf, pool_pr, pool_py, cfg, x0,
):
    """Software-pipelined sweep over N_TILES batch tiles ("subs").

    Emission per iteration s:  fc(s) -> CP(s-1) -> ELU-y(s-1) ->
    fc_out(s-2) -> ELU-x chain(s) [ACT/DVE].  The PE stream (fc, CP,
    fc_out of three consecutive subs) then never waits on the elementwise
    chains in steady state.  fc_out PSUM tiles ride the pru/prv rings
    (free during the fc_out phase), keeping total PSUM at 8 banks:
    pf 1x2 + pru 2 + prv 2 + py 2.
    """
    import concourse.bass as bass
    from concourse import mybir

    Ident = mybir.ActivationFunctionType.Identity
    idt, mdt, odt = dts
    compute_only = cfg.get("compute_only", False)  # timing diagnostic
    n_subs = N_TILES * reps
    G = cfg["in_db"]  # subs per input DMA group
    GO = cfg["out_db"]  # subs per output DMA group
    n_grp = N_TILES // G
    state = {"x": {0: x0}, "p12": {}, "a12": {}, "ye": {}, "osb": {}}

    def load(g):
        if compute_only:
            state["x"][g] = x0
            return
        bsl = bass.ds((g % n_grp) * G * NT, G * NT)
        x1sb = pool_in.tile([128, 4, G * NT], idt, tag="x1")
        nc.sync.dma_start(out=x1sb, in_=x1v[:, :, bsl])
        x2sb = pool_in.tile([128, 4, G * NT], idt, tag="x2")
        nc.sync.dma_start(out=x2sb, in_=x2v[:, :, bsl])
        state["x"][g] = (x1sb, x2sb)

    def fc(s):
        x1sb, x2sb = state["x"][s // G]
        bsl = slice((s % G) * NT, (s % G + 1) * NT)
        p12 = pool_pf.tile([128, 2 * NT], f32, tag="pf")
        for half, (xsb, wsb) in enumerate(((x1sb, w1sb), (x2sb, w2sb))):
            for k in range(4):
                nc.tensor.matmul(
                    p12[:, half * NT : (half + 1) * NT],
                    lhsT=wsb[:, k, :],
                    rhs=xsb[:, k, bsl],
                    start=(k == 0),
                    stop=(k == 3),
                )
        if s % G == G - 1:
            state["x"].pop(s // G)
        state["p12"][s] = p12

    def chain(s):
        # a' = ELU(p12)+1 = relu(p12) + min(exp(p12), 1).  Emitted LAST in
        # each iteration: it depends on fc(s), and putting it ahead of the
        # (s-1)/(s-2) stage ops would head-of-line-block the in-order
        # ACT/DVE queues while the PE waits on tt(s-1).
        p12 = state["p12"].pop(s)
        e12 = pool_tmp.tile([128, 2 * NT], idt, tag="e12")
        nc.scalar.activation(e12, p12, Exp)
        m12 = pool_tmp.tile([128, 2 * NT], idt, tag="m12")
        nc.vector.tensor_scalar(m12, e12, 1.0, None, Alu.min)
        a12 = pool_a.tile([128, 2 * NT], mdt, tag="a12")
        nc.vector.scalar_tensor_tensor(
            a12, in0=p12, scalar=0.0, in1=m12, op0=Alu.max, op1=Alu.add
        )
        state["a12"][s] = a12

    def cp(s):
        # tt[(g,r)] = (U.a1 - su) * (V.a2 - sv);  y = Wc.T @ tt
        a12 = state["a12"].pop(s)
        a1 = a12[:, :NT]
        a2 = a12[:, NT:]
        yp = pool_py.tile([128, NT], f32, tag="py")
        for p in range(2):
            ru = pool_pr.tile([128, NT], f32, tag="pru")
            nc.tensor.matmul(ru, lhsT=eusb[:, p, :], rhs=a1)
            # stage (ru - su) into SBUF on ACT: DVE reads at most one PSUM
            # operand per op.
            ttf = pool_t.tile([128, NT], mdt, tag="ttf")
            nc.scalar.activation(ttf, ru, Ident, bias=susb[:, p : p + 1])
            rv = pool_pr.tile([128, NT], f32, tag="prv")
            nc.tensor.matmul(rv, lhsT=evsb[:, p, :], rhs=a2)
            tt = pool_t.tile([128, NT], mdt, tag="tt")
            nc.vector.scalar_tensor_tensor(
                tt, in0=rv, scalar=svsb[:, p : p + 1], in1=ttf,
                op0=Alu.subtract, op1=Alu.mult,
            )
            nc.tensor.matmul(
                yp, lhsT=wcsb[:, p, :], rhs=tt,
                start=(p == 0), stop=(p == 1),
            )
        # ye = ELU(yp)+1 (the -1 is folded into the fc_out bias)
        ey = pool_tmp.tile([128, NT], idt, tag="ey")
        nc.scalar.activation(ey, yp, Exp)
        my = pool_tmp.tile([128, NT], idt, tag="my")
        nc.vector.tensor_scalar(my, ey, 1.0, None, Alu.min)
        ye = pool_ye.tile([128, NT], idt, tag="ye")
        nc.vector.scalar_tensor_tensor(
            ye, in0=yp, scalar=0.0, in1=my, op0=Alu.max, op1=Alu.add
        )
        state["ye"][s] = ye

    def fcout(s):
        if s % GO == 0:
            osb_new = pool_out.tile([128, 4, GO * NT], odt, tag="osb")
            state["osb"][s // GO] = osb_new
        outsb = state["osb"][s // GO]
        ye = state["ye"].pop(s)
        ys = slice((s % GO) * NT, (s % GO + 1) * NT)
        # alternate the ACT/DVE split of the PSUM->SBUF bias+converts to
        # balance both engines at ~2.5/1.5 per sub
        dve_c = (3,) if s % 2 == 0 else (1, 3)
        for c in range(4):
            o = pool_pr.tile([128, NT], f32, tag=("pru" if c % 2 == 0 else "prv"))
            nc.tensor.matmul(o, lhsT=woutsb[:, c, :], rhs=ye)
            dst = outsb[:, c, ys]
            if c in dve_c:
                nc.vector.tensor_scalar(
                    dst, o, boutsb[:, c : c + 1], None, Alu.add
                )
            else:
                nc.scalar.add(dst, o, boutsb[:, c : c + 1])
        if s % GO == GO - 1:
            outsb = state["osb"].pop(s // GO)
            if not compute_only or s == n_subs - 1:
                bsl = bass.ds(((s // GO) % (N_TILES // GO)) * GO * NT, GO * NT)
                # stores go out on the ACT HWDGE queue so they don't queue
                # behind the input loads on the Sync queue
                nc.scalar.dma_start(out=outv[:, :, bsl], in_=outsb)

    for s in range(n_subs + 2):
        if s % G == 0 and 0 < s // G + 1 < (n_subs + G - 1) // G:
            load(s // G + 1)
        if s < n_subs:
            fc(s)
        if 1 <= s <= n_subs:
            cp(s - 1)
        if s >= 2:
            fcout(s - 2)
        if s < n_subs:
            chain(s)


def _cp_decompose(Wb, R=8, seeds=8, iters=2500, target=1e-11):
    """Batched CP-ALS over all 32 groups: Wb[g,o,i,j] = sum_r C[g,o,r] U[g,i,r] V[g,j,r].
    Deterministic (fixed seeds). Returns (C, U, V, max_rel_err)."""
    T = Wb.astype(np.float64)  # [G, O, I, J]
    G, O, I, J = T.shape
    normT = np.linalg.norm(T.reshape(G, -1), axis=1)  # [G]

    bestC = np.zeros((G, O, R))
    bestU = np.zeros((G, I, R))
    bestV = np.zeros((G, J, R))
    best_err = np.full(G, np.inf)

    def solve(Tmat, KR):
        # Tmat [G, D, IJ], KR [G, IJ, R] -> X [G, D, R] minimizing ||Tmat - X KR^T||
        Gm = KR.transpose(0, 2, 1) @ KR  # [G, R, R]
        Gm = Gm + 1e-13 * np.eye(R)[None]
        rhs = Tmat @ KR  # [G, D, R]
        return np.linalg.solve(Gm, rhs.transpose(0, 2, 1)).transpose(0, 2, 1)

    for seed in range(seeds):
        active = best_err > target
        if not active.any():
            break
        rng = np.random.default_rng(1234 + seed)
        C = rng.standard_normal((G, O, R))
        U = rng.standard_normal((G, I, R))
        V = rng.standard_normal((G, J, R))
        for _ in range(iters):
            KR = (U[:, :, None, :] * V[:, None, :, :]).reshape(G, I * J, R)
            C = solve(T.reshape(G, O, I * J), KR)
            KR = (C[:, :, None, :] * V[:, None, :, :]).reshape(G, O * J, R)
            U = solve(T.transpose(0, 2, 1, 3).reshape(G, I, O * J), KR)
            KR = (C[:, :, None, :] * U[:, None, :, :]).reshape(G, O * I, R)
            V = solve(T.transpose(0, 3, 1, 2).reshape(G, J, O * I), KR)
            nc_ = np.linalg.norm(C, axis=1)
            nu = np.linalg.norm(U, axis=1)
            nv = np.linalg.norm(V, axis=1)
            s = (nc_ * nu * nv) ** (1.0 / 3.0)
            C *= (s / np.maximum(nc_, 1e-300))[:, None, :]
            U *= (s / np.maximum(nu, 1e-300))[:, None, :]
            V *= (s / np.maximum(nv, 1e-300))[:, None, :]
        rec = np.einsum("gor,gir,gjr->goij", C, U, V)
        err = np.linalg.norm((rec - T).reshape(G, -1), axis=1) / normT
        take = (err < best_err) & active
        bestC[take], bestU[take], bestV[take] = C[take], U[take], V[take]
        best_err[take] = err[take]
    return bestC, bestU, bestV, float(best_err.max())


def _np_dt(name):
    if name == "bfloat16":
        import ml_dtypes

        return np.dtype(ml_dtypes.bfloat16)
    if name == "float16":
        return np.dtype(np.float16)
    return np.dtype(np.float32)


def _make_consts(W1, W2, Wout, Wb, cfg):
    """Host-side constant matrices for the device program."""
    fi = _np_dt(cfg["io_dtype"])
    fm = _np_dt(cfg["cp_dtype"])
    f = np.float32
    # lhsT chunks for fc1/fc2: [K=feat chunk, M=internal]
    w1t = np.stack([W1[:, k * 128 : (k + 1) * 128].T for k in range(4)]).astype(fi)
    w2t = np.stack([W2[:, k * 128 : (k + 1) * 128].T for k in range(4)]).astype(fi)
    # lhsT chunks for fc_out: [K=internal, M=out chunk]
    woutt = np.stack([Wout[c * 128 : (c + 1) * 128, :].T for c in range(4)]).astype(fi)

    # CP decomposition of the per-group bilinear tensors (R=8 is exact for
    # generic 4x4x4): y[g,o] = sum_r C[g,o,r] (U[g,:,r].x1g) (V[g,:,r].x2g)
    R = 8
    C, U, V, cp_err = _cp_decompose(Wb, R=R)
    # CP-space row layout: chunk p (<2) holds groups [16p, 16p+16), row
    # m = g_loc*8 + r.
    eu = np.zeros((2, 128, 128), f)  # lhsT: [k=(g*4+i), m=(g_loc*8+r)]
    ev = np.zeros((2, 128, 128), f)
    wcm = np.zeros((2, 128, 128), f)  # lhsT: [k=(g_loc*8+r), m=(g*4+o)]
    suc = np.zeros((128, 2), f)
    svc = np.zeros((128, 2), f)
    for p in range(2):
        for gl in range(16):
            g = p * 16 + gl
            for r in range(R):
                m = gl * 8 + r
                eu[p, g * 4 : g * 4 + 4, m] = U[g, :, r]
                ev[p, g * 4 : g * 4 + 4, m] = V[g, :, r]
                wcm[p, m, g * 4 : g * 4 + 4] = C[g, :, r]
                # ttf staging runs on ACT as Identity(ru + bias): store -su
                suc[m, p] = -U[g, :, r].sum()
                svc[m, p] = V[g, :, r].sum()

    # fc_out bias: out = Wout @ (ye' - 1) = Wout@ye' - Wout@1
    bvec = -Wout.astype(np.float64).sum(axis=1)
    bout = np.stack([bvec[c * 128 : (c + 1) * 128] for c in range(4)], axis=1).astype(f)

    return dict(
        w1t=w1t, w2t=w2t, woutt=woutt,
        eu=eu.astype(fm), ev=ev.astype(fm), wc=wcm.astype(fm),
        suc=suc, svc=svc, bout=bout,
    )


def kernel(input1, input2, W1, W2, Wout, Wb):
    _ensure_path()
    from concourse.bass_utils import run_bass_kernel_spmd

    cfg = dict(DEFAULT_CFG, **_CACHE.get("cfg_override", {}))
    if "nc" not in _CACHE:
        _CACHE["nc"] = _build_program(cfg=cfg)
    nc = _CACHE["nc"]

    W1, W2, Wout, Wb = (np.asarray(a) for a in (W1, W2, Wout, Wb))
    ckey = (W1.tobytes()[:64], Wb.tobytes()[:256])
    if _CACHE.get("ckey") != ckey:
        _CACHE["consts"] = _make_consts(W1, W2, Wout, Wb, cfg)
        _CACHE["ckey"] = ckey
    consts = _CACHE["consts"]
    fi = _np_dt(cfg["io_dtype"])
    x1b = np.asarray(input1).astype(fi)
    x2b = np.asarray(input2).astype(fi)

    in_maps = []
    for c in range(N_CORES):
        sl = slice(c * B_CORE, (c + 1) * B_CORE)
        m = dict(consts)
        m["x1t"] = np.ascontiguousarray(x1b[sl].T)
        m["x2t"] = np.ascontiguousarray(x2b[sl].T)
        in_maps.append(m)

    res = run_bass_kernel_spmd(nc, in_maps, list(range(N_CORES)))
    _CACHE["last_res"] = res

    out = np.empty((B, OUT), np.float32)
    for c in range(N_CORES):
        out[c * B_CORE : (c + 1) * B_CORE, :] = res.results[c]["outt"].T.astype(
            np.float32
        )
    return out


# revision 45
# speedup vs baseline: 1.1973x; 1.1973x over previous
"""Trainium2 Bass kernel for nn_AggregatedBilinear.

Computation (per batch row b):
    x1 = ELU(input1 @ W1.T)                    # [128]
    x2 = ELU(input2 @ W2.T)                    # [128]
    y[g,o] = sum_ij x1[g,i] Wb[g,o,i,j] x2[g,j]   (g<32, o,i,j<4)
    out = ELU(y) @ Wout.T                      # [512]

Strategy: data-parallel over 8 NeuronCores (8192 batch rows each). On-chip
layout is feature-major (features on SBUF partitions, batch on the free dim),
so the host pre-transposes each input shard (fp32 DMA-transpose does not
exist on trn2) and transposes the result back. All DMA'd tensors are bf16
(halves HBM traffic; the correctness budget is rel RMS < 2e-2).

The per-group bilinear uses a rank-8 CP decomposition (exact for these
4x4x4 tensors):
    Wb[g,o,i,j] = sum_r C[g,o,r] U[g,i,r] V[g,j,r]
    tt[(g,r)]   = (U.T a1)[(g,r)] * (V.T a2)[(g,r)]    # PE + DVE multiply
    y           = Wc.T @ tt                             # PE, 2 chunks of 128

ELU is computed in the shifted form a' = ELU(x)+1 = relu(x) + min(exp(x),1)
(min(exp(x),1) == exp(min(x,0)) including exp overflow to +inf, so the
unclamped ACT Exp is safe).  The +1 shifts are folded into constants:
ttf = ru - su with su = U.T 1, tt = (rv - sv)*ttf, and the fc_out bias
bout = -Wout @ 1.  Engine assignment per batch tile keeps every engine
under the ~76us DMA floor: ACT does the Exps + ru staging, GpSimd (no
PSUM port, so SBUF-only ops) does the min's, DVE does the 2-stream
merges, and the PSUM->SBUF output converts alternate ACT/DVE.
"""

import numpy as np

B = 65536
IN1 = IN2 = 512
OUT = 512
CARD = 32
WIDTH = 4
INTERNAL = CARD * WIDTH  # 128
N_CORES = 8
B_CORE = B // N_CORES  # 8192
NT = 512  # batch columns per tile (one PSUM bank)
N_TILES = B_CORE // NT  # 16

_CACHE = {}


def _ensure_path():
    import sys

    try:
        import concourse  # noqa: F401
    except ImportError:
        for p in ("/opt/trn_rl_repo", "/root/.axon_site/_ro/trn_rl_repo"):
            if p not in sys.path:
                sys.path.insert(0, p)


def _split_excess_waits(nc, max_waits=1):
    """walrus CoreV3 codegen rejects instructions with more than a couple of
    semaphore waits; split excess waits onto preceding NoOps."""
    from concourse import mybir

    n_new = 0
    for f in nc.m.functions:
        for bb in f.blocks:
            insts = list(bb.instructions)
            out = []
            changed = False
            for inst in insts:
                si = inst.sync_info
                if si is not None and si.on_wait and len(si.on_wait) > max_waits:
                    waits = list(si.on_wait)
                    excess, keep = waits[:-max_waits], waits[-max_waits:]
                    for i in range(0, len(excess), max_waits):
                        nop = mybir.InstNoOp(
                            name=f"waitsplit-{n_new}",
                            engine=inst.engine,
                            ins=[],
                            outs=[],
                            sync_info=mybir.SyncInfo(
                                on_wait=excess[i : i + max_waits], on_update=[]
                            ),
                        )
                        n_new += 1
                        out.append(nop)
                    inst.sync_info = mybir.SyncInfo(
                        on_wait=keep, on_update=list(si.on_update or [])
                    )
                    changed = True
                out.append(inst)
            if changed:
                bb.instructions[:] = out
    return n_new


DEFAULT_CFG = dict(
    in_db=2, out_db=2, bufs_in=2, bufs_pf=1, bufs_pr=2, bufs_py=2,
    bufs_t=6, bufs_ye=3, bufs_out=2,
    io_dtype="float16", cp_dtype="float16", out_dtype="float16",
)


def _build_program(reps=1, loop_reps=None, cfg=None):
    """loop_reps: wrap the whole batch sweep in an on-device For_i that runs
    it loop_reps times (same I/O; used only for timing measurements)."""
    cfg = dict(DEFAULT_CFG, **(cfg or {}))
    import concourse.bass as bass
    import concourse.tile as tile
    from concourse import mybir

    f32 = mybir.dt.float32
    # idt: dtype of the DMA'd inputs + fc1/fc2/fc_out matmul operands.
    # mdt: dtype of the CP-stage matmul operands (tt and eu/ev/wc).
    # odt: dtype of the DMA'd output.
    idt = getattr(mybir.dt, cfg["io_dtype"])
    mdt = getattr(mybir.dt, cfg["cp_dtype"])
    odt = getattr(mybir.dt, cfg["out_dtype"])
    Exp = mybir.ActivationFunctionType.Exp
    Alu = mybir.AluOpType

    nc = bass.Bass()
    x1t = nc.declare_dram_parameter("x1t", [IN1, B_CORE], idt, isOutput=False)
    x2t = nc.declare_dram_parameter("x2t", [IN2, B_CORE], idt, isOutput=False)
    w1t = nc.declare_dram_parameter("w1t", [4, 128, 128], idt, isOutput=False)
    w2t = nc.declare_dram_parameter("w2t", [4, 128, 128], idt, isOutput=False)
    woutt = nc.declare_dram_parameter("woutt", [4, 128, 128], idt, isOutput=False)
    eu = nc.declare_dram_parameter("eu", [2, 128, 128], mdt, isOutput=False)
    ev = nc.declare_dram_parameter("ev", [2, 128, 128], mdt, isOutput=False)
    wc = nc.declare_dram_parameter("wc", [2, 128, 128], mdt, isOutput=False)
    suc = nc.declare_dram_parameter("suc", [128, 2], f32, isOutput=False)
    svc = nc.declare_dram_parameter("svc", [128, 2], f32, isOutput=False)
    bout = nc.declare_dram_parameter("bout", [128, 4], f32, isOutput=False)
    outt = nc.declare_dram_parameter("outt", [OUT, B_CORE], odt, isOutput=True)

    with tile.TileContext(nc) as tc:
        with (
            tc.tile_pool(name="consts", bufs=1) as consts,
            tc.tile_pool(name="inp", bufs=cfg["bufs_in"]) as pool_in,
            tc.tile_pool(name="inp0", bufs=1) as pool_in0,
            tc.tile_pool(name="tmp", bufs=2) as pool_tmp,
            tc.tile_pool(name="act", bufs=2) as pool_a,
            tc.tile_pool(name="tmul", bufs=cfg["bufs_t"]) as pool_t,
            tc.tile_pool(name="yep", bufs=cfg["bufs_ye"]) as pool_ye,
            tc.tile_pool(name="outs", bufs=cfg["bufs_out"]) as pool_out,
            tc.tile_pool(name="pfc", bufs=cfg["bufs_pf"], space="PSUM") as pool_pf,
            tc.tile_pool(name="prep", bufs=cfg["bufs_pr"], space="PSUM") as pool_pr,
            tc.tile_pool(name="py", bufs=cfg["bufs_py"], space="PSUM") as pool_py,
        ):
            # --- first input group DMAs issue before the (many) const DMAs
            # so the x transfers start streaming immediately.  The first
            # sub's two tiles go first as small separate transfers so fc(0)
            # can start after ~1/G of the group transfer time. ---
            G = cfg["in_db"]
            x1v0 = x1t.rearrange("(c k) b -> k c b", k=128)
            x2v0 = x2t.rearrange("(c k) b -> k c b", k=128)
            x1sb0 = pool_in0.tile([128, 4, G * NT], idt, tag="x10")
            x2sb0 = pool_in0.tile([128, 4, G * NT], idt, tag="x20")
            sl0 = bass.ds(0, NT)
            nc.sync.dma_start(out=x1sb0[:, :, sl0], in_=x1v0[:, :, sl0])
            nc.sync.dma_start(out=x2sb0[:, :, sl0], in_=x2v0[:, :, sl0])

            # --- constants into SBUF ---
            def load_const3(h, dt, nchunk=4):
                t = consts.tile([128, nchunk, 128], dt, tag=h.name)
                nc.sync.dma_start(out=t, in_=h.rearrange("c k m -> k c m"))
                return t

            # fc weights right behind sub-0's inputs so fc(0) starts early
            w1sb = load_const3(w1t, idt)
            w2sb = load_const3(w2t, idt)
            for sub in range(1, G):
                sl = bass.ds(sub * NT, NT)
                nc.sync.dma_start(out=x1sb0[:, :, sl], in_=x1v0[:, :, sl])
                nc.sync.dma_start(out=x2sb0[:, :, sl], in_=x2v0[:, :, sl])
            woutsb = load_const3(woutt, idt)
            eusb = load_const3(eu, mdt, 2)
            evsb = load_const3(ev, mdt, 2)
            wcsb = load_const3(wc, mdt, 2)
            susb = consts.tile([128, 2], f32, tag="suc")
            nc.sync.dma_start(out=susb, in_=suc[:, :])
            svsb = consts.tile([128, 2], f32, tag="svc")
            nc.sync.dma_start(out=svsb, in_=svc[:, :])
            boutsb = consts.tile([128, 4], f32, tag="bout")
            nc.sync.dma_start(out=boutsb, in_=bout[:, :])

            x1v = x1t.rearrange("(c k) b -> k c b", k=128)
            x2v = x2t.rearrange("(c k) b -> k c b", k=128)
            outv = outt.rearrange("(c k) b -> k c b", k=128)

            import contextlib

            loop_cm = (
                tc.For_i(0, loop_reps, 1)
                if loop_reps is not None
                else contextlib.nullcontext()
            )
            with loop_cm:
                _batch_sweep(
                    nc, tc, reps, f32, (idt, mdt, odt), Exp, Alu,
                    x1v, x2v, outv,
                    w1sb, w2sb, woutsb, eusb, evsb, wcsb,
                    susb, svsb, boutsb,
                    pool_in, pool_tmp, pool_a, pool_t, pool_ye, pool_out,
                    pool_pf, pool_pr, pool_py, cfg,
                    (x1sb0, x2sb0),
                )

    _split_excess_waits(nc)
    return nc


def _batch_sweep(
    nc, tc, reps, f32, dts, Exp, Alu,
    x1v, x2v, outv,
    w1sb, w2sb, woutsb, eusb, evsb, wcsb,
    susb, svsb, boutsb,
    pool_in, pool_tmp, pool_a, pool_t, pool_ye, pool_out,
    pool_pf, pool_pr, pool_py, cfg, x0,
):
    """Software-pipelined sweep over N_TILES batch tiles ("subs").

    Emission per iteration s:  fc(s) -> CP(s-1) -> ELU-y(s-1) ->
    fc_out(s-2) -> ELU-x chain(s) [ACT/DVE].  The PE stream (fc, CP,
    fc_out of three consecutive subs) then never waits on the elementwise
    chains in steady state.  fc_out PSUM tiles ride the pru/prv rings
    (free during the fc_out phase), keeping total PSUM at 8 banks:
    pf 1x2 + pru 2 + prv 2 + py 2.
    """
    import concourse.bass as bass
    from concourse import mybir

    Ident = mybir.ActivationFunctionType.Identity
    idt, mdt, odt = dts
    compute_only = cfg.get("compute_only", False)  # timing diagnostic
    n_subs = N_TILES * reps
    G = cfg["in_db"]  # subs per input DMA group
    GO = cfg["out_db"]  # subs per output DMA group
    n_grp = N_TILES // G
    state = {"x": {0: x0}, "p12": {}, "a12": {}, "ye": {}, "osb": {}}

    def load(g):
        if compute_only:
            state["x"][g] = x0
            return
        bsl = bass.ds((g % n_grp) * G * NT, G * NT)
        x1sb = pool_in.tile([128, 4, G * NT], idt, tag="x1")
        nc.sync.dma_start(out=x1sb, in_=x1v[:, :, bsl])
        x2sb = pool_in.tile([128, 4, G * NT], idt, tag="x2")
        nc.sync.dma_start(out=x2sb, in_=x2v[:, :, bsl])
        state["x"][g] = (x1sb, x2sb)

    def fc(s):
        x1sb, x2sb = state["x"][s // G]
        bsl = slice((s % G) * NT, (s % G + 1) * NT)
        p12 = pool_pf.tile([128, 2 * NT], f32, tag="pf")
        for half, (xsb, wsb) in enumerate(((x1sb, w1sb), (x2sb, w2sb))):
            for k in range(4):
                nc.tensor.matmul(
                    p12[:, half * NT : (half + 1) * NT],
                    lhsT=wsb[:, k, :],
                    rhs=xsb[:, k, bsl],
                    start=(k == 0),
                    stop=(k == 3),
                )
        if s % G == G - 1:
            state["x"].pop(s // G)
        state["p12"][s] = p12

    def chain(s):
        # a' = ELU(p12)+1 = relu(p12) + min(exp(p12), 1).  Emitted LAST in
        # each iteration: it depends on fc(s), and putting it ahead of the
        # (s-1)/(s-2) stage ops would head-of-line-block the in-order
        # ACT/DVE queues while the PE waits on tt(s-1).
        p12 = state["p12"].pop(s)
        e12 = pool_tmp.tile([128, 2 * NT], idt, tag="e12")
        nc.scalar.activation(e12, p12, Exp)
        m12 = pool_tmp.tile([128, 2 * NT], idt, tag="m12")
        nc.vector.tensor_scalar(m12, e12, 1.0, None, Alu.min)
        a12 = pool_a.tile([128, 2 * NT], mdt, tag="a12")
        nc.vector.scalar_tensor_tensor(
            a12, in0=p12, scalar=0.0, in1=m12, op0=Alu.max, op1=Alu.add
        )
        state["a12"][s] = a12

    def cp(s):
        # tt[(g,r)] = (U.a1 - su) * (V.a2 - sv);  y = Wc.T @ tt
        a12 = state["a12"].pop(s)
        a1 = a12[:, :NT]
        a2 = a12[:, NT:]
        yp = pool_py.tile([128, NT], f32, tag="py")
        for p in range(2):
            ru = pool_pr.tile([128, NT], f32, tag="pru")
            nc.tensor.matmul(ru, lhsT=eusb[:, p, :], rhs=a1)
            # stage (ru - su) into SBUF on ACT: DVE reads at most one PSUM
            # operand per op.
            ttf = pool_t.tile([128, NT], mdt, tag="ttf")
            nc.scalar.activation(ttf, ru, Ident, bias=susb[:, p : p + 1])
            rv = pool_pr.tile([128, NT], f32, tag="prv")
            nc.tensor.matmul(rv, lhsT=evsb[:, p, :], rhs=a2)
            tt = pool_t.tile([128, NT], mdt, tag="tt")
            nc.vector.scalar_tensor_tensor(
                tt, in0=rv, scalar=svsb[:, p : p + 1], in1=ttf,
                op0=Alu.subtract, op1=Alu.mult,
            )
            nc.tensor.matmul(
                yp, lhsT=wcsb[:, p, :], rhs=tt,
                start=(p == 0), stop=(p == 1),
            )
        # ye = ELU(yp)+1 (the -1 is folded into the fc_out bias)
        ey = pool_tmp.tile([128, NT], idt, tag="ey")
        nc.scalar.activation(ey, yp, Exp)
        my = pool_tmp.tile([128, NT], idt, tag="my")
        nc.vector.tensor_scalar(my, ey, 1.0, None, Alu.min)
        ye = pool_ye.tile([128, NT], idt, tag="ye")
        nc.vector.scalar_tensor_tensor(
            ye, in0=yp, scalar=0.0, in1=my, op0=Alu.max, op1=Alu.add
        )
        state["ye"][s] = ye

    def fcout(s):
        if s % GO == 0:
            osb_new = pool_out.tile([128, 4, GO * NT], odt, tag="osb")
            state["osb"][s // GO] = osb_new
        outsb = state["osb"][s // GO]
        ye = state["ye"].pop(s)
        ys = slice((s % GO) * NT, (s % GO + 1) * NT)
        # alternate the ACT/DVE split of the PSUM->SBUF bias+converts to
        # balance both engines at ~2.5/1.5 per sub
        dve_c = (3,) if s % 2 == 0 else (1, 3)
        for c in range(4):
            o = pool_pr.tile([128, NT], f32, tag=("pru" if c % 2 == 0 else "prv"))
            nc.tensor.matmul(o, lhsT=woutsb[:, c, :], rhs=ye)
            dst = outsb[:, c, ys]
            if c in dve_c:
                nc.vector.tensor_scalar(
                    dst, o, boutsb[:, c : c + 1], None, Alu.add
                )
            else:
                nc.scalar.add(dst, o, boutsb[:, c : c + 1])
        if s % GO == GO - 1:
            outsb = state["osb"].pop(s // GO)
            if not compute_only or s == n_subs - 1:
                bsl = bass.ds(((s // GO) % (N_TILES // GO)) * GO * NT, GO * NT)
                nc.sync.dma_start(out=outv[:, :, bsl], in_=outsb)

    for s in range(n_subs + 2):
        if s % G == 0 and 0 < s // G + 1 < (n_subs + G - 1) // G:
            load(s // G + 1)
        if s < n_subs:
            fc(s)
        if 1 <= s <= n_subs:
            cp(s - 1)
        if s >= 2:
            fcout(s - 2)
        if s < n_subs:
            chain(s)


def _cp_decompose(Wb, R=8, seeds=8, iters=2500, target=1e-11):
    """Batched CP-ALS over all 32 groups: Wb[g,o,i,j] = sum_r C[g,o,r] U[g,i,r] V[g,j,r].
    Deterministic (fixed seeds). Returns (C, U, V, max_rel_err)."""
    T = Wb.astype(np.float64)  # [G, O, I, J]
    G, O, I, J = T.shape
    normT = np.linalg.norm(T.reshape(G, -1), axis=1)  # [G]

    bestC = np.zeros((G, O, R))
    bestU = np.zeros((G, I, R))
    bestV = np.zeros((G, J, R))
    best_err = np.full(G, np.inf)

    def solve(Tmat, KR):
        # Tmat [G, D, IJ], KR [G, IJ, R] -> X [G, D, R] minimizing ||Tmat - X KR^T||
        Gm = KR.transpose(0, 2, 1) @ KR  # [G, R, R]
        Gm = Gm + 1e-13 * np.eye(R)[None]
        rhs = Tmat @ KR  # [G, D, R]
        return np.linalg.solve(Gm, rhs.transpose(0, 2, 1)).transpose(0, 2, 1)

    for seed in range(seeds):
        active = best_err > target
        if not active.any():
            break
        rng = np.random.default_rng(1234 + seed)
        C = rng.standard_normal((G, O, R))
        U = rng.standard_normal((G, I, R))
        V = rng.standard_normal((G, J, R))
        for _ in range(iters):
            KR = (U[:, :, None, :] * V[:, None, :, :]).reshape(G, I * J, R)
            C = solve(T.reshape(G, O, I * J), KR)
            KR = (C[:, :, None, :] * V[:, None, :, :]).reshape(G, O * J, R)
            U = solve(T.transpose(0, 2, 1, 3).reshape(G, I, O * J), KR)
            KR = (C[:, :, None, :] * U[:, None, :, :]).reshape(G, O * I, R)
            V = solve(T.transpose(0, 3, 1, 2).reshape(G, J, O * I), KR)
            nc_ = np.linalg.norm(C, axis=1)
            nu = np.linalg.norm(U, axis=1)
            nv = np.linalg.norm(V, axis=1)
            s = (nc_ * nu * nv) ** (1.0 / 3.0)
            C *= (s / np.maximum(nc_, 1e-300))[:, None, :]
            U *= (s / np.maximum(nu, 1e-300))[:, None, :]
            V *= (s / np.maximum(nv, 1e-300))[:, None, :]
        rec = np.einsum("gor,gir,gjr->goij", C, U, V)
        err = np.linalg.norm((rec - T).reshape(G, -1), axis=1) / normT
        take = (err < best_err) & active
        bestC[take], bestU[take], bestV[take] = C[take], U[take], V[take]
        best_err[take] = err[take]
    return bestC, bestU, bestV, float(best_err.max())


def _np_dt(name):
    if name == "bfloat16":
        import ml_dtypes

        return np.dtype(ml_dtypes.bfloat16)
    if name == "float16":
        return np.dtype(np.float16)
    return np.dtype(np.float32)


def _make_consts(W1, W2, Wout, Wb, cfg):
    """Host-side constant matrices for the device program."""
    fi = _np_dt(cfg["io_dtype"])
    fm = _np_dt(cfg["cp_dtype"])
    f = np.float32
    # lhsT chunks for fc1/fc2: [K=feat chunk, M=internal]
    w1t = np.stack([W1[:, k * 128 : (k + 1) * 128].T for k in range(4)]).astype(fi)
    w2t = np.stack([W2[:, k * 128 : (k + 1) * 128].T for k in range(4)]).astype(fi)
    # lhsT chunks for fc_out: [K=internal, M=out chunk]
    woutt = np.stack([Wout[c * 128 : (c + 1) * 128, :].T for c in range(4)]).astype(fi)

    # CP decomposition of the per-group bilinear tensors (R=8 is exact for
    # generic 4x4x4): y[g,o] = sum_r C[g,o,r] (U[g,:,r].x1g) (V[g,:,r].x2g)
    R = 8
    C, U, V, cp_err = _cp_decompose(Wb, R=R)
    # CP-space row layout: chunk p (<2) holds groups [16p, 16p+16), row
    # m = g_loc*8 + r.
    eu = np.zeros((2, 128, 128), f)  # lhsT: [k=(g*4+i), m=(g_loc*8+r)]
    ev = np.zeros((2, 128, 128), f)
    wcm = np.zeros((2, 128, 128), f)  # lhsT: [k=(g_loc*8+r), m=(g*4+o)]
    suc = np.zeros((128, 2), f)
    svc = np.zeros((128, 2), f)
    for p in range(2):
        for gl in range(16):
            g = p * 16 + gl
            for r in range(R):
                m = gl * 8 + r
                eu[p, g * 4 : g * 4 + 4, m] = U[g, :, r]
                ev[p, g * 4 : g * 4 + 4, m] = V[g, :, r]
                wcm[p, m, g * 4 : g * 4 + 4] = C[g, :, r]
                # ttf staging runs on ACT as Identity(ru + bias): store -su
                suc[m, p] = -U[g, :, r].sum()
                svc[m, p] = V[g, :, r].sum()

    # fc_out bias: out = Wout @ (ye' - 1) = Wout@ye' - Wout@1
    bvec = -Wout.astype(np.float64).sum(axis=1)
    bout = np.stack([bvec[c * 128 : (c + 1) * 128] for c in range(4)], axis=1).astype(f)

    return dict(
        w1t=w1t, w2t=w2t, woutt=woutt,
        eu=eu.astype(fm), ev=ev.astype(fm), wc=wcm.astype(fm),
        suc=suc, svc=svc, bout=bout,
    )


def kernel(input1, input2, W1, W2, Wout, Wb):
    _ensure_path()
    from concourse.bass_utils import run_bass_kernel_spmd

    cfg = dict(DEFAULT_CFG, **_CACHE.get("cfg_override", {}))
    if "nc" not in _CACHE:
        _CACHE["nc"] = _build_program(cfg=cfg)
    nc = _CACHE["nc"]

    W1, W2, Wout, Wb = (np.asarray(a) for a in (W1, W2, Wout, Wb))
    ckey = (W1.tobytes()[:64], Wb.tobytes()[:256])
    if _CACHE.get("ckey") != ckey:
        _CACHE["consts"] = _make_consts(W1, W2, Wout, Wb, cfg)
        _CACHE["ckey"] = ckey
    consts = _CACHE["consts"]
    fi = _np_dt(cfg["io_dtype"])
    x1b = np.asarray(input1).astype(fi)
    x2b = np.asarray(input2).astype(fi)

    in_maps = []
    for c in range(N_CORES):
        sl = slice(c * B_CORE, (c + 1) * B_CORE)
        m = dict(consts)
        m["x1t"] = np.ascontiguousarray(x1b[sl].T)
        m["x2t"] = np.ascontiguousarray(x2b[sl].T)
        in_maps.append(m)

    res = run_bass_kernel_spmd(nc, in_maps, list(range(N_CORES)))
    _CACHE["last_res"] = res

    out = np.empty((B, OUT), np.float32)
    for c in range(N_CORES):
        out[c * B_CORE : (c + 1) * B_CORE, :] = res.results[c]["outt"].T.astype(
            np.float32
        )
    return out
